# revision 3
# baseline (speedup 1.0000x reference)
"""Message-passing kernel for Trainium2 (8 NeuronCores, data-parallel over batch).

Full bf16 datapath, contiguous carries, rhs-shifted taps into a fixed
psum region, HAM-warm fillers, early-start DMA order, PE pre-warm.

Reference computation (per batch element, C=128 channels, H=128, W=256):
  4 sequential directional scans (down, up, right, left); each scan step is
    out[i] = x[i] + relu(conv1d(out[i-1]))
  with a 'same'-padded K=9 conv1d (C->C) along the non-scan spatial axis.

Design (per core, one batch element):
  - image resident in SBUF as bf16 [C=128 partitions, H*264]
    (per-row cols 0-3 / 260-263 zero guards for the row-scan taps)
  - all matmuls bf16 (1 cycle/row at any width; FWL halves weight loads):
    each conv tap shifts the *rhs* start by its offset s and accumulates
    into a fixed psum region (no psum-offset alignment games)
  - row scans: rhs = image row, 256-wide taps
  - col scans: a colbuf [C, 256*136] of contiguous per-column slots
    (4 guard + 128 data + 4 guard).  Phase 3 (right) reads carry from slot
    w-1, writes slot w; +x comes from image columns prefetched to small
    contiguous tiles by ScalarE.  Phase 4 (left) reads carry from slot w+1,
    +x directly from slot w (contiguous), overwrites slot w in place, and
    the finished 32-column blocks stream to DRAM (w-major; host undoes).
  - recurrence update x + relu(psum) fused into one DVE
    scalar_tensor_tensor (the only engine that can read PSUM and add a
    tensor; GpSimd cannot access PSUM, ScalarE cannot add a tensor)
  - filler matmuls into a scratch psum bank keep the PE HAM-warm (2.4 GHz)
    through each step's update/semaphore window
"""

import numpy as np

C = 128
H = 128
W = 256
K = 9
RS = 264          # image row stride (bf16): 4 guard + 256 data + 4 guard
SS = 136          # colbuf slot stride: 4 guard + 128 data + 4 guard
B = 8
N_CORES = 8
SBLK = 32         # output staging block (columns)
N_FILL = 3
DVE_ROW = 144     # DVE half of the row update (Pool gets 256-DVE_ROW)
DVE_COL = 72      # DVE half of the col update (Pool gets 128-DVE_COL)

_CACHE = {}


# ---------------------------------------------------------------------------
# workarounds for this walrus build (exit drain / per-instruction wait limits)
# ---------------------------------------------------------------------------

def _patch_tile_drain():
    import concourse.mybir as mybir
    import concourse.tile as tile_mod
    from concourse.vector_clock import ScopedClock

    def _drain_and_barrier(self, tick_clock, wait_clock):
        nc = self.nc
        probe = nc.sync.nop()
        wait_clock.add_sem_waits(
            probe.ins, ScopedClock({None: tick_clock.global_clock})
        )
        si = probe.ins.sync_info
        waits = list(si.on_wait) if si is not None else []
        if si is not None:
            probe.ins.sync_info = mybir.SyncInfo(
                on_wait=[], on_update=list(si.on_update)
            )
        for w in waits:
            wi = nc.sync.nop()
            wi.ins.sync_info = mybir.SyncInfo(on_wait=[w], on_update=[])
        nc.sync.drain()

        nc.all_engine_barrier()
        assert self.sems is not None
        popped = nc._tile_sem_poison_stack.pop()
        assert popped is self._sem_poison
        nc.clear_and_free_semaphores(list(self.sems.allocated().values()))
        nc.all_engine_barrier()

    tile_mod.TileContext._drain_and_barrier = _drain_and_barrier


def _split_waits(nc, max_waits=1):
    """This walrus build allows only one semaphore wait per instruction;
    move excess waits onto nops inserted just before, same engine.  Keep a
    PE-updated semaphore (typically the psum producer, last to arrive) on
    the instruction itself so the chained-nop latency hides behind it."""
    import concourse.mybir as mybir

    ctr = 0
    for f in nc.m.functions:
        for bb in f.blocks:
            insts = bb.instructions
            if not any(
                i.sync_info is not None and len(i.sync_info.on_wait) > max_waits
                for i in insts
            ):
                continue
            new = []
            for inst in insts:
                si = inst.sync_info
                ws = list(si.on_wait) if si is not None else []
                if len(ws) > max_waits:
                    ws.sort(key=lambda w: "PE" in (w.ant_name or ""))
                    extra, keep = ws[:-max_waits], ws[-max_waits:]
                    for j in range(0, len(extra), max_waits):
                        ctr += 1
                        nop = mybir.InstNoOp(
                            name=f"waitsplit-{ctr}",
                            sync_info=mybir.SyncInfo(
                                on_wait=extra[j:j + max_waits], on_update=[]
                            ),
                            bass_nofuse=True,
                            engine=inst.engine,
                        )
                        new.append(nop)
                    inst.sync_info = mybir.SyncInfo(
                        on_wait=keep, on_update=list(si.on_update)
                    )
                new.append(inst)
            bb.instructions = new


# ---------------------------------------------------------------------------
# program construction
# ---------------------------------------------------------------------------

def _build_program(n_fill=N_FILL):
    import concourse.bass as bass
    import concourse.mybir as mybir
    from concourse.alu_op_type import AluOpType
    from concourse.tile import TileContext

    _patch_tile_drain()

    f32 = mybir.dt.float32
    bf16 = mybir.dt.bfloat16
    u32 = mybir.dt.uint32

    nc = bass.Bass()
    x_in = nc.declare_dram_parameter("x", [C, H * W], bf16, isOutput=False)
    w_in = {}
    for nm in ("wd", "wu", "wr", "wl"):
        w_in[nm] = nc.declare_dram_parameter(nm, [C, K * C], bf16, isOutput=False)
    # w-major output: y[c, w*H + h]; host transposes back
    y_out = nc.declare_dram_parameter("y", [C, W * H], bf16, isOutput=True)
    y3 = y_out.rearrange("p (w h) -> p w h", h=H)

    with TileContext(nc) as tc:
        with (
            tc.tile_pool(name="img", bufs=1) as imgp,
            tc.tile_pool(name="cbuf", bufs=1) as cbp,
            tc.tile_pool(name="wpool", bufs=1) as wp,
            tc.tile_pool(name="xcp", bufs=1) as xcp,
            tc.tile_pool(name="psum", bufs=4, space="PSUM") as pp,
            tc.tile_pool(name="fpsum", bufs=2, space="PSUM") as fp,
        ):
            # DMA order: phase-1 weights, first x rows, then the rest, so
            # the first scan step starts as early as possible
            wt = {}
            for nm in ("wd", "wu", "wr", "wl"):
                wt[nm] = wp.tile([C, K * C], bf16, tag=f"wt_{nm}", name=f"wt_{nm}")
            nc.sync.dma_start(out=wt["wd"][:], in_=w_in["wd"][:])

            img = imgp.tile([C, H * RS], bf16, tag="img")
            img3 = img.rearrange("p (h r) -> p h r", r=RS)
            # zero row guards
            nc.vector.memset(img3[:, :, 0:4].bitcast(u32), 0)
            nc.vector.memset(img3[:, :, 260:264].bitcast(u32), 0)
            x3 = x_in.rearrange("p (h w) -> p h w", w=W)
            nc.sync.dma_start(out=img3[:, 0:16, 4:260], in_=x3[:, 0:16, :])
            for nm in ("wu", "wr", "wl"):
                nc.sync.dma_start(out=wt[nm][:], in_=w_in[nm][:])
            for hb in range(16, H, 16):
                nc.sync.dma_start(
                    out=img3[:, hb:hb + 16, 4:260], in_=x3[:, hb:hb + 16, :]
                )

            # column buffer: 256 slots of SS; only the guards need zeroing
            # (data region is fully written by phase 3).  On ScalarE, which
            # is idle through the row scans.
            cbuf = cbp.tile([C, W * SS], bf16, tag="cbuf")
            cb3 = cbuf.rearrange("p (w r) -> p w r", r=SS)
            nc.scalar.memzero(cb3[:, :, 0:4])
            nc.scalar.memzero(cb3[:, :, 132:136])

            # prefetched +x columns for phase 3
            xcols = [
                xcp.tile([C, C], bf16, tag=f"xc{ci}", name=f"xc{ci}")
                for ci in range(4)
            ]

            filler_rhs = wt["wd"][:, 0:256]

            def prewarm(n):
                # junk matmuls on the already-loaded wd weights, issued
                # before the first scan step to warm the PE clock gate
                fps = fp.tile([C, 256], f32, tag="fps")
                for fi in range(n):
                    nc.tensor.matmul(
                        fps[:], wt["wd"][:, (fi % K) * C:(fi % K + 1) * C],
                        filler_rhs, start=(fi == 0), stop=(fi == n - 1),
                    )

            prewarm(40)

            def row(i):
                return img3[:, i, :]

            def col(w):
                # image column w as a strided AP (ScalarE copies only)
                return img3[:, :, 4 + w]

            def slot(w):
                return cb3[:, w, :]

            def row_taps(wtile, r, ps):
                # ps [C, 256]; out[j] = ps[j]
                for t in range(K):
                    s = t - 4
                    nc.tensor.matmul(
                        ps[:, 0:256], wtile[:, t * C:(t + 1) * C],
                        r[:, 4 + s:260 + s],
                        start=(t == 0), stop=(t == K - 1),
                    )

            def col_taps(wtile, carry, ps):
                # carry: slot AP [C, SS]; ps [C, 128]; out[j] = ps[j]
                for t in range(K):
                    s = t - 4
                    nc.tensor.matmul(
                        ps[:, 0:128], wtile[:, t * C:(t + 1) * C],
                        carry[:, 4 + s:132 + s],
                        start=(t == 0), stop=(t == K - 1),
                    )

            def fillers(n):
                if not n:
                    return
                fps = fp.tile([C, 256], f32, tag="fps")
                for fi in range(n):
                    nc.tensor.matmul(
                        fps[:], wt["wd"][:, fi * C:(fi + 1) * C], filler_rhs,
                        start=(fi == 0), stop=(fi == n - 1),
                    )

            def stt(out, in0, in1):
                # out = max(in0, 0) + in1.  DVE only: it is the single
                # engine that can both read PSUM and do tensor+tensor.
                nc.vector.scalar_tensor_tensor(
                    out=out, in0=in0, scalar=0.0, in1=in1,
                    op0=AluOpType.max, op1=AluOpType.add,
                )

            # ---------------- phase 1 down / phase 2 up --------------------
            for phase, wname, order in (
                (1, "wd", range(1, H)),
                (2, "wu", range(H - 2, -1, -1)),
            ):
                src_off = -1 if phase == 1 else 1
                for i in order:
                    ps = pp.tile([C, 256], f32, tag="ps")
                    row_taps(wt[wname], row(i + src_off), ps)
                    stt(row(i)[:, 4:260], ps[:, 0:256], row(i)[:, 4:260])
                    fillers(n_fill)

            # ---------------- phase 3: right -------------------------------
            # seed: slot 0 = phase-2 column 0; prefetch +x for w=1,2
            nc.scalar.copy(out=slot(0)[:, 4:132], in_=col(0))
            nc.scalar.copy(out=xcols[1][:], in_=col(1))
            nc.scalar.copy(out=xcols[2][:], in_=col(2))
            for w in range(1, W):
                ps = pp.tile([C, 128], f32, tag="ps")
                col_taps(wt["wr"], slot(w - 1), ps)
                stt(slot(w)[:, 4:132], ps[:, 0:128], xcols[w % 4][:])
                if w + 2 < W:
                    nc.scalar.copy(out=xcols[(w + 2) % 4][:], in_=col(w + 2))
                fillers(n_fill)

            # ---------------- phase 4: left (stores overlap) ---------------
            def flush(b):
                nc.sync.dma_start(
                    out=y3[:, b * SBLK:(b + 1) * SBLK, :],
                    in_=cb3[:, b * SBLK:(b + 1) * SBLK, 4:132],
                )

            for w in range(W - 2, -1, -1):
                ps = pp.tile([C, 128], f32, tag="ps")
                col_taps(wt["wl"], slot(w + 1), ps)
                stt(slot(w)[:, 4:132], ps[:, 0:128], slot(w)[:, 4:132])
                if w == 8:
                    nc.sync.dma_start(
                        out=y3[:, 8:SBLK, :], in_=cb3[:, 8:SBLK, 4:132]
                    )
                elif w == 0:
                    nc.sync.dma_start(
                        out=y3[:, 0:8, :], in_=cb3[:, 0:8, 4:132]
                    )
                elif w % SBLK == 0:
                    flush(w // SBLK)
                fillers(n_fill)

    _split_waits(nc, max_waits=1)
    return nc


def _get_program():
    key = "prog"
    if key not in _CACHE:
        _CACHE[key] = _build_program()
    return _CACHE[key]


# ---------------------------------------------------------------------------
# entry point
# ---------------------------------------------------------------------------

def kernel(x, w_down, w_up, w_right, w_left, _trace=False):
    import ml_dtypes
    from concourse.bass_utils import run_bass_kernel_spmd

    bf16 = ml_dtypes.bfloat16
    nc = _get_program()

    def prep_w(w):
        # w: (Cout, Cin, K) -> lhsT layout [Cin, K*Cout]
        return np.ascontiguousarray(
            np.transpose(np.asarray(w, np.float32), (1, 2, 0)).reshape(C, K * C)
        ).astype(bf16)

    wd, wu, wr, wl = (prep_w(w) for w in (w_down, w_up, w_right, w_left))
    x = np.asarray(x, np.float32).astype(bf16)
    in_maps = [
        {
            "x": np.ascontiguousarray(x[b].reshape(C, H * W)),
            "wd": wd, "wu": wu, "wr": wr, "wl": wl,
        }
        for b in range(B)
    ]
    res = run_bass_kernel_spmd(
        nc, in_maps, list(range(N_CORES)), trace=_trace
    )
    # y is w-major [C, W*H]; transpose back to [C, H, W]
    out = np.stack(
        [
            res.results[b]["y"].astype(np.float32).reshape(C, W, H).transpose(0, 2, 1)
            for b in range(B)
        ]
    )
    if _trace:
        return out, res
    return out
